# revision 17
# baseline (speedup 1.0000x reference)
"""Multi-head attention (B=8, N=1024, D=768, H=12) on 8 TRN2 NeuronCores.

Strategy: data-parallel over batch — core b computes the full attention for
batch element b. No collectives. Inside a core everything is laid out so no
on-chip transposes are needed:

  - host feeds x[b] pre-transposed as xT [768, 1024] (feature-major, bf16)
  - qT = W_q-as-lhsT @ xT   -> [768, 1024]  (feature-major, per-head rows)
  - kT likewise; v = x @ W_v  -> [1024, 768] (token-major), stored per-head
    as [ki, head, 65] with a ones column at col 64
  - S^T tile = kT_h-slice @ qT_h -> [ki, qi]: softmax-exp is elementwise on
    ACT, and the PV matmul consumes exp(S^T) directly as the moving operand:
      out^T[d(+denom), qi] = v_aug.T @ E — the ones column makes row 64 the
    softmax denominator, accumulated in the same PSUM tile.
  - normalize via reciprocal + partition-broadcast (DRAM-bounce DMA) multiply
  - proj uses attn_out^T tiles as stationary, W_proj natural-layout moving.

Matmuls run in bf16 (inputs rounded host-side / on-chip), accumulation fp32.
"""

import numpy as np
import ml_dtypes

B = 8
N = 1024
D = 768
H = 12
HD = 64
P = 128
DT = 6  # number of 128-row tiles in D
NT = 8  # number of 128-row tiles in N
NCORES = 8

BF16 = ml_dtypes.bfloat16

_CACHE = {}


def _emit(nc, tc, tile, mybir, dram, rep=0):
    import contextlib

    import concourse.bass as bass

    f32 = mybir.dt.float32
    bf16 = mybir.dt.bfloat16
    Exp = mybir.ActivationFunctionType.Exp

    xT_d, wqkv_d, bqk_d, bvb_d, wproj_d, bpb_d, out_d = dram
    # DRAM bounce for partition-broadcasting softmax reciprocal denominators
    # (SBUF->SBUF DMA cannot read with partition step 0; DRAM can).
    rds_d = nc.dram_tensor(f"rdscratch{rep}", [DT, 4, 512], f32)

    ctx = contextlib.ExitStack()
    with ctx:
        persist = ctx.enter_context(tc.tile_pool(name="persist", bufs=1))
        wqk_pool = ctx.enter_context(tc.tile_pool(name="wqk", bufs=4))
        wbig_pool = ctx.enter_context(tc.tile_pool(name="wbig", bufs=2))
        e_pool = ctx.enter_context(tc.tile_pool(name="e", bufs=3))
        bc_pool = ctx.enter_context(tc.tile_pool(name="bc", bufs=4))
        rd_pool = ctx.enter_context(tc.tile_pool(name="rd", bufs=4))
        ost_pool = ctx.enter_context(tc.tile_pool(name="ost", bufs=3))
        # PSUM budget (8 banks): qkv/proj accumulators 2 + S^T double-buffer
        # 2x[128,1024]=4 + PV accumulators 2.
        ps = ctx.enter_context(tc.tile_pool(name="ps", bufs=2, space="PSUM"))
        av = ctx.enter_context(tc.tile_pool(name="av", bufs=2, space="PSUM"))
        sps = ctx.enter_context(tc.tile_pool(name="sps", bufs=1, space="PSUM"))

        xT_t = persist.tile([P, DT, N], bf16)
        qT_t = persist.tile([P, DT, N], bf16)
        kT_t = persist.tile([P, DT, N], bf16)
        vst_t = persist.tile([P, NT, H, HD + 1], bf16)
        aT_t = persist.tile([P, DT, N], bf16)
        bqk_t = persist.tile([P, 2 * DT], f32)
        bvb_t = persist.tile([P, D], f32)
        bpb_t = persist.tile([P, D], f32)

        # input loads
        nc.sync.dma_start(out=xT_t[:], in_=xT_d.ap().rearrange("(i p) n -> p i n", p=P))
        nc.sync.dma_start(out=bqk_t[:], in_=bqk_d[:, :])
        nc.sync.dma_start(out=bvb_t[:], in_=bvb_d[:, :])
        nc.sync.dma_start(out=bpb_t[:], in_=bpb_d[:, :])
        wv_t = wbig_pool.tile([P, DT, D], bf16, tag="wbig")
        nc.sync.dma_start(
            out=wv_t[:],
            in_=wqkv_d.ap()[:, 2 * D : 3 * D].rearrange("(i p) f -> p i f", p=P),
        )
        wp_t = wbig_pool.tile([P, DT, D], bf16, tag="wbig", name="wp_t")
        nc.sync.dma_start(
            out=wp_t[:], in_=wproj_d.ap().rearrange("(i p) f -> p i f", p=P)
        )
        nc.vector.memset(vst_t[:, :, :, HD : HD + 1], 1.0)

        # ---- V phase: v = x @ W_v + b_v, scattered into [ki, head, 65] ----
        for nt in range(NT):
            for c0, cw in ((0, 512), (512, 256)):
                pst = ps.tile([P, 512], f32, tag="ps")
                for d in range(DT):
                    nc.tensor.matmul(
                        pst[:, :cw],
                        lhsT=xT_t[:, d, nt * P : (nt + 1) * P],
                        rhs=wv_t[:, d, c0 : c0 + cw],
                        start=(d == 0),
                        stop=(d == DT - 1),
                    )
                nc.vector.tensor_add(
                    out=vst_t[:, nt, c0 // HD : (c0 + cw) // HD, 0:HD],
                    in0=pst[:, :cw].rearrange("p (h c) -> p h c", c=HD),
                    in1=bvb_t[:, c0 : c0 + cw].rearrange("p (h c) -> p h c", c=HD),
                )

        # ---- Q/K phase: qT/kT = (x @ W_{q,k})^T + b, feature-major ----
        for pr in range(DT):
            for woff, boff, dstT in ((0, 0, qT_t), (D, DT, kT_t)):
                wt = wqk_pool.tile([P, DT, P], bf16, tag="wqk", name=f"wt{pr}_{boff}")
                nc.sync.dma_start(
                    out=wt[:],
                    in_=wqkv_d.ap()[
                        :, woff + pr * P : woff + (pr + 1) * P
                    ].rearrange("(i p) f -> p i f", p=P),
                )
                for nh in range(2):
                    pst = ps.tile([P, 512], f32, tag="ps")
                    for d in range(DT):
                        nc.tensor.matmul(
                            pst[:],
                            lhsT=wt[:, d, :],
                            rhs=xT_t[:, d, nh * 512 : (nh + 1) * 512],
                            start=(d == 0),
                            stop=(d == DT - 1),
                        )
                    nc.vector.tensor_scalar_add(
                        out=dstT[:, pr, nh * 512 : (nh + 1) * 512],
                        in0=pst[:],
                        scalar1=bqk_t[:, pr + boff : pr + boff + 1],
                    )

        # ---- attention: head-pair (2*pr, 2*pr+1), one qi-half at a time ----
        # S^T matmuls for the two heads land on disjoint PE row groups
        # (partitions 0:64 / 64:128) so they run concurrently on the array.
        for pr in range(DT):
            for nh in range(2):
                qs = slice(nh * 512, (nh + 1) * 512)
                avts = [
                    av.tile([P, 512], f32, tag="av", name=f"avt{pr}_{nh}_{j}")
                    for j in range(2)
                ]
                for kk in range(NT // 2):
                    # two ki-tiles per S^T PSUM tile -> one 2048-wide exp call
                    spt = sps.tile([P, 2048], f32, tag="sps")
                    for t in range(2):
                        for j, hp in enumerate((0, HD)):  # A/B on alt row groups
                            ki = 2 * kk + t
                            nc.tensor.matmul(
                                spt[:, (2 * j + t) * 512 : (2 * j + t + 1) * 512],
                                lhsT=kT_t[hp : hp + HD, pr, ki * P : (ki + 1) * P],
                                rhs=qT_t[hp : hp + HD, pr, qs],
                                start=True,
                                stop=True,
                            )
                    et = e_pool.tile([P, 2048], bf16, tag="e")
                    nc.scalar.activation(out=et[:], in_=spt[:], func=Exp, scale=0.125)
                    for t in range(2):
                        for j in range(2):
                            ki = 2 * kk + t
                            nc.tensor.matmul(
                                avts[j][0 : HD + 1, :],
                                lhsT=vst_t[:, ki, 2 * pr + j, :],
                                rhs=et[:, (2 * j + t) * 512 : (2 * j + t + 1) * 512],
                                start=(ki == 0),
                                stop=(ki == NT - 1),
                            )
                for j in range(2):
                    hp = j * HD
                    rdt = rd_pool.tile([1, 512], f32, tag="rd")
                    nc.vector.reciprocal(rdt[0:1, :], avts[j][HD : HD + 1, :])
                    nc.sync.dma_start(
                        out=rds_d.ap()[pr, 2 * nh + j, :], in_=rdt[0:1, :]
                    )
                    bct = bc_pool.tile([HD, 512], f32, tag="bc")
                    src = rds_d.ap()[pr, 2 * nh + j, :]
                    src_b = bass.AP(
                        tensor=src.tensor,
                        offset=src.offset,
                        ap=[[0, HD]] + list(src.ap),
                    )
                    nc.sync.dma_start(out=bct[:], in_=src_b)
                    nc.vector.tensor_mul(
                        out=aT_t[hp : hp + HD, pr, qs],
                        in0=avts[j][0:HD, :],
                        in1=bct[:, :],
                    )

        # ---- proj: out = attn_out @ W_proj + b_proj ----
        for nt in range(NT):
            ot = ost_pool.tile([P, D], f32, tag="ost")
            for c0, cw in ((0, 512), (512, 256)):
                pst = ps.tile([P, 512], f32, tag="ps")
                for d in range(DT):
                    nc.tensor.matmul(
                        pst[:, :cw],
                        lhsT=aT_t[:, d, nt * P : (nt + 1) * P],
                        rhs=wp_t[:, d, c0 : c0 + cw],
                        start=(d == 0),
                        stop=(d == DT - 1),
                    )
                nc.vector.tensor_add(
                    out=ot[:, c0 : c0 + cw],
                    in0=pst[:, :cw],
                    in1=bpb_t[:, c0 : c0 + cw],
                )
            nc.sync.dma_start(out=out_d.ap()[nt * P : (nt + 1) * P, :], in_=ot[:])


def build_module(reps=1, barrier=False):
    import concourse.mybir as mybir
    import concourse.tile as tile
    from concourse import bacc

    f32 = mybir.dt.float32
    bf16 = mybir.dt.bfloat16
    nc = bacc.Bacc(None, target_bir_lowering=False)
    xT_d = nc.dram_tensor("xT", [D, N], bf16, kind="ExternalInput")
    wqkv_d = nc.dram_tensor("w_qkv", [D, 3 * D], bf16, kind="ExternalInput")
    bqk_d = nc.dram_tensor("b_qk", [P, 2 * DT], f32, kind="ExternalInput")
    bvb_d = nc.dram_tensor("b_v_bc", [P, D], f32, kind="ExternalInput")
    wproj_d = nc.dram_tensor("w_proj", [D, D], bf16, kind="ExternalInput")
    bpb_d = nc.dram_tensor("b_proj_bc", [P, D], f32, kind="ExternalInput")
    out_d = nc.dram_tensor("out", [N, D], f32, kind="ExternalOutput")

    with tile.TileContext(nc) as tc:
        for rep in range(reps):
            if barrier and rep:
                tc.strict_bb_all_engine_barrier()
            _emit(
                nc,
                tc,
                tile,
                mybir,
                (xT_d, wqkv_d, bqk_d, bvb_d, wproj_d, bpb_d, out_d),
                rep=rep,
            )
    nc.compile()
    return nc


def get_nc():
    if "nc" not in _CACHE:
        _CACHE["nc"] = build_module()
    return _CACHE["nc"]


def shard_inputs(x, W_qkv, b_qkv, W_proj, b_proj):
    """Host-side layout/dtype prep + per-core sharding (batch data-parallel)."""
    x = np.asarray(x, dtype=np.float32)
    W_qkv = np.asarray(W_qkv, dtype=np.float32)
    b_qkv = np.asarray(b_qkv, dtype=np.float32)
    W_proj = np.asarray(W_proj, dtype=np.float32)
    b_proj = np.asarray(b_proj, dtype=np.float32)

    xT = np.ascontiguousarray(x.transpose(0, 2, 1)).astype(BF16)  # [B, D, N]
    wqkv = W_qkv.astype(BF16)
    wproj = W_proj.astype(BF16)
    bqk = np.ascontiguousarray(b_qkv[: 2 * D].reshape(2 * DT, P).T)  # [P, 12]
    bvb = np.ascontiguousarray(np.broadcast_to(b_qkv[2 * D :], (P, D)))
    bpb = np.ascontiguousarray(np.broadcast_to(b_proj, (P, D)))
    return [
        {
            "xT": xT[b],
            "w_qkv": wqkv,
            "b_qk": bqk,
            "b_v_bc": bvb,
            "w_proj": wproj,
            "b_proj_bc": bpb,
        }
        for b in range(NCORES)
    ]


def kernel(x, W_qkv, b_qkv, W_proj, b_proj):
    from concourse.bass_utils import run_bass_kernel_spmd

    nc = get_nc()
    in_maps = shard_inputs(x, W_qkv, b_qkv, W_proj, b_proj)
    res = run_bass_kernel_spmd(nc, in_maps, list(range(NCORES)))
    out = np.stack([res.results[b]["out"] for b in range(NCORES)], axis=0)
    return out.astype(np.float32)


# revision 19
# speedup vs baseline: 1.0050x; 1.0050x over previous
"""Multi-head attention (B=8, N=1024, D=768, H=12) on 8 TRN2 NeuronCores.

Strategy: data-parallel over batch — core b computes the full attention for
batch element b. No collectives. Inside a core everything is laid out so no
on-chip transposes are needed:

  - host feeds x[b] pre-transposed as xT [768, 1024] (feature-major, bf16)
  - qT = W_q-as-lhsT @ xT   -> [768, 1024]  (feature-major, per-head rows)
  - kT likewise; v = x @ W_v  -> [1024, 768] (token-major), stored per-head
    as [ki, head, 65] with a ones column at col 64
  - S^T tile = kT_h-slice @ qT_h -> [ki, qi]: softmax-exp is elementwise on
    ACT, and the PV matmul consumes exp(S^T) directly as the moving operand:
      out^T[d(+denom), qi] = v_aug.T @ E — the ones column makes row 64 the
    softmax denominator, accumulated in the same PSUM tile.
  - normalize via reciprocal + partition-broadcast (DRAM-bounce DMA) multiply
  - proj uses attn_out^T tiles as stationary, W_proj natural-layout moving.

Matmuls run in bf16 (inputs rounded host-side / on-chip), accumulation fp32.
"""

import numpy as np
import ml_dtypes

B = 8
N = 1024
D = 768
H = 12
HD = 64
P = 128
DT = 6  # number of 128-row tiles in D
NT = 8  # number of 128-row tiles in N
NCORES = 8

BF16 = ml_dtypes.bfloat16

_CACHE = {}


def _emit(nc, tc, tile, mybir, dram, rep=0):
    import contextlib

    import concourse.bass as bass

    f32 = mybir.dt.float32
    bf16 = mybir.dt.bfloat16
    Exp = mybir.ActivationFunctionType.Exp

    xT_d, wqkv_d, bqk_d, bvb_d, wproj_d, bpb_d, out_d = dram
    # DRAM bounce for partition-broadcasting softmax reciprocal denominators
    # (SBUF->SBUF DMA cannot read with partition step 0; DRAM can).
    rds_d = nc.dram_tensor(f"rdscratch{rep}", [DT, 4, 512], f32)

    ctx = contextlib.ExitStack()
    with ctx:
        persist = ctx.enter_context(tc.tile_pool(name="persist", bufs=1))
        wqk_pool = ctx.enter_context(tc.tile_pool(name="wqk", bufs=4))
        wbig_pool = ctx.enter_context(tc.tile_pool(name="wbig", bufs=2))
        e_pool = ctx.enter_context(tc.tile_pool(name="e", bufs=3))
        bc_pool = ctx.enter_context(tc.tile_pool(name="bc", bufs=4))
        rd_pool = ctx.enter_context(tc.tile_pool(name="rd", bufs=4))
        ost_pool = ctx.enter_context(tc.tile_pool(name="ost", bufs=3))
        # PSUM budget (8 banks): qkv/proj accumulators 2 + S^T double-buffer
        # 2x[128,1024]=4 + PV accumulators 2.
        ps = ctx.enter_context(tc.tile_pool(name="ps", bufs=2, space="PSUM"))
        av = ctx.enter_context(tc.tile_pool(name="av", bufs=2, space="PSUM"))
        sps = ctx.enter_context(tc.tile_pool(name="sps", bufs=2, space="PSUM"))

        xT_t = persist.tile([P, DT, N], bf16)
        qT_t = persist.tile([P, DT, N], bf16)
        kT_t = persist.tile([P, DT, N], bf16)
        vst_t = persist.tile([P, NT, H, HD + 1], bf16)
        aT_t = persist.tile([P, DT, N], bf16)
        bqk_t = persist.tile([P, 2 * DT], f32)
        bvb_t = persist.tile([P, D], f32)
        bpb_t = persist.tile([P, D], f32)

        # input loads
        nc.sync.dma_start(out=xT_t[:], in_=xT_d.ap().rearrange("(i p) n -> p i n", p=P))
        nc.sync.dma_start(out=bqk_t[:], in_=bqk_d[:, :])
        nc.sync.dma_start(out=bvb_t[:], in_=bvb_d[:, :])
        nc.sync.dma_start(out=bpb_t[:], in_=bpb_d[:, :])
        wv_t = wbig_pool.tile([P, DT, D], bf16, tag="wbig")
        nc.sync.dma_start(
            out=wv_t[:],
            in_=wqkv_d.ap()[:, 2 * D : 3 * D].rearrange("(i p) f -> p i f", p=P),
        )
        wp_t = wbig_pool.tile([P, DT, D], bf16, tag="wbig", name="wp_t")
        nc.sync.dma_start(
            out=wp_t[:], in_=wproj_d.ap().rearrange("(i p) f -> p i f", p=P)
        )
        nc.vector.memset(vst_t[:, :, :, HD : HD + 1], 1.0)

        # ---- V phase: v = x @ W_v + b_v, scattered into [ki, head, 65] ----
        for nt in range(NT):
            for c0, cw in ((0, 512), (512, 256)):
                pst = ps.tile([P, 512], f32, tag="ps")
                for d in range(DT):
                    nc.tensor.matmul(
                        pst[:, :cw],
                        lhsT=xT_t[:, d, nt * P : (nt + 1) * P],
                        rhs=wv_t[:, d, c0 : c0 + cw],
                        start=(d == 0),
                        stop=(d == DT - 1),
                    )
                nc.vector.tensor_add(
                    out=vst_t[:, nt, c0 // HD : (c0 + cw) // HD, 0:HD],
                    in0=pst[:, :cw].rearrange("p (h c) -> p h c", c=HD),
                    in1=bvb_t[:, c0 : c0 + cw].rearrange("p (h c) -> p h c", c=HD),
                )

        # ---- Q/K phase: qT/kT = (x @ W_{q,k})^T + b, feature-major ----
        for pr in range(DT):
            for woff, boff, dstT in ((0, 0, qT_t), (D, DT, kT_t)):
                wt = wqk_pool.tile([P, DT, P], bf16, tag="wqk", name=f"wt{pr}_{boff}")
                nc.sync.dma_start(
                    out=wt[:],
                    in_=wqkv_d.ap()[
                        :, woff + pr * P : woff + (pr + 1) * P
                    ].rearrange("(i p) f -> p i f", p=P),
                )
                for nh in range(2):
                    pst = ps.tile([P, 512], f32, tag="ps")
                    for d in range(DT):
                        nc.tensor.matmul(
                            pst[:],
                            lhsT=wt[:, d, :],
                            rhs=xT_t[:, d, nh * 512 : (nh + 1) * 512],
                            start=(d == 0),
                            stop=(d == DT - 1),
                        )
                    nc.vector.tensor_scalar_add(
                        out=dstT[:, pr, nh * 512 : (nh + 1) * 512],
                        in0=pst[:],
                        scalar1=bqk_t[:, pr + boff : pr + boff + 1],
                    )

        # ---- attention: head-pair (2*pr, 2*pr+1), one qi-half at a time ----
        # S^T matmuls for the two heads land on disjoint PE row groups
        # (partitions 0:64 / 64:128) so they run concurrently on the array.
        for pr in range(DT):
            for nh in range(2):
                qs = slice(nh * 512, (nh + 1) * 512)
                avts = [
                    av.tile([P, 512], f32, tag="av", name=f"avt{pr}_{nh}_{j}")
                    for j in range(2)
                ]
                for ki in range(NT):
                    spt = sps.tile([P, 1024], f32, tag="sps")
                    for j, hp in enumerate((0, HD)):  # A/B on alt row groups
                        nc.tensor.matmul(
                            spt[:, j * 512 : (j + 1) * 512],
                            lhsT=kT_t[hp : hp + HD, pr, ki * P : (ki + 1) * P],
                            rhs=qT_t[hp : hp + HD, pr, qs],
                            start=True,
                            stop=True,
                        )
                    et = e_pool.tile([P, 1024], bf16, tag="e")
                    nc.scalar.activation(out=et[:], in_=spt[:], func=Exp, scale=0.125)
                    for j in range(2):
                        nc.tensor.matmul(
                            avts[j][0 : HD + 1, :],
                            lhsT=vst_t[:, ki, 2 * pr + j, :],
                            rhs=et[:, j * 512 : (j + 1) * 512],
                            start=(ki == 0),
                            stop=(ki == NT - 1),
                        )
                for j in range(2):
                    hp = j * HD
                    rdt = rd_pool.tile([1, 512], f32, tag="rd")
                    nc.vector.reciprocal(rdt[0:1, :], avts[j][HD : HD + 1, :])
                    nc.sync.dma_start(
                        out=rds_d.ap()[pr, 2 * nh + j, :], in_=rdt[0:1, :]
                    )
                    bct = bc_pool.tile([HD, 512], f32, tag="bc")
                    src = rds_d.ap()[pr, 2 * nh + j, :]
                    src_b = bass.AP(
                        tensor=src.tensor,
                        offset=src.offset,
                        ap=[[0, HD]] + list(src.ap),
                    )
                    nc.sync.dma_start(out=bct[:], in_=src_b)
                    nc.vector.tensor_mul(
                        out=aT_t[hp : hp + HD, pr, qs],
                        in0=avts[j][0:HD, :],
                        in1=bct[:, :],
                    )

        # ---- proj: out = attn_out @ W_proj + b_proj ----
        for nt in range(NT):
            ot = ost_pool.tile([P, D], f32, tag="ost")
            for c0, cw in ((0, 512), (512, 256)):
                pst = ps.tile([P, 512], f32, tag="ps")
                for d in range(DT):
                    nc.tensor.matmul(
                        pst[:, :cw],
                        lhsT=aT_t[:, d, nt * P : (nt + 1) * P],
                        rhs=wp_t[:, d, c0 : c0 + cw],
                        start=(d == 0),
                        stop=(d == DT - 1),
                    )
                nc.vector.tensor_add(
                    out=ot[:, c0 : c0 + cw],
                    in0=pst[:, :cw],
                    in1=bpb_t[:, c0 : c0 + cw],
                )
            nc.sync.dma_start(out=out_d.ap()[nt * P : (nt + 1) * P, :], in_=ot[:])


def build_module(reps=1, barrier=False):
    import concourse.mybir as mybir
    import concourse.tile as tile
    from concourse import bacc

    f32 = mybir.dt.float32
    bf16 = mybir.dt.bfloat16
    nc = bacc.Bacc(None, target_bir_lowering=False)
    xT_d = nc.dram_tensor("xT", [D, N], bf16, kind="ExternalInput")
    wqkv_d = nc.dram_tensor("w_qkv", [D, 3 * D], bf16, kind="ExternalInput")
    bqk_d = nc.dram_tensor("b_qk", [P, 2 * DT], f32, kind="ExternalInput")
    bvb_d = nc.dram_tensor("b_v_bc", [P, D], f32, kind="ExternalInput")
    wproj_d = nc.dram_tensor("w_proj", [D, D], bf16, kind="ExternalInput")
    bpb_d = nc.dram_tensor("b_proj_bc", [P, D], f32, kind="ExternalInput")
    out_d = nc.dram_tensor("out", [N, D], f32, kind="ExternalOutput")

    with tile.TileContext(nc) as tc:
        for rep in range(reps):
            if barrier and rep:
                tc.strict_bb_all_engine_barrier()
            _emit(
                nc,
                tc,
                tile,
                mybir,
                (xT_d, wqkv_d, bqk_d, bvb_d, wproj_d, bpb_d, out_d),
                rep=rep,
            )
    nc.compile()
    return nc


def get_nc():
    if "nc" not in _CACHE:
        _CACHE["nc"] = build_module()
    return _CACHE["nc"]


def shard_inputs(x, W_qkv, b_qkv, W_proj, b_proj):
    """Host-side layout/dtype prep + per-core sharding (batch data-parallel)."""
    x = np.asarray(x, dtype=np.float32)
    W_qkv = np.asarray(W_qkv, dtype=np.float32)
    b_qkv = np.asarray(b_qkv, dtype=np.float32)
    W_proj = np.asarray(W_proj, dtype=np.float32)
    b_proj = np.asarray(b_proj, dtype=np.float32)

    xT = np.ascontiguousarray(x.transpose(0, 2, 1)).astype(BF16)  # [B, D, N]
    wqkv = W_qkv.astype(BF16)
    wproj = W_proj.astype(BF16)
    bqk = np.ascontiguousarray(b_qkv[: 2 * D].reshape(2 * DT, P).T)  # [P, 12]
    bvb = np.ascontiguousarray(np.broadcast_to(b_qkv[2 * D :], (P, D)))
    bpb = np.ascontiguousarray(np.broadcast_to(b_proj, (P, D)))
    return [
        {
            "xT": xT[b],
            "w_qkv": wqkv,
            "b_qk": bqk,
            "b_v_bc": bvb,
            "w_proj": wproj,
            "b_proj_bc": bpb,
        }
        for b in range(NCORES)
    ]


def kernel(x, W_qkv, b_qkv, W_proj, b_proj):
    from concourse.bass_utils import run_bass_kernel_spmd

    nc = get_nc()
    in_maps = shard_inputs(x, W_qkv, b_qkv, W_proj, b_proj)
    res = run_bass_kernel_spmd(nc, in_maps, list(range(NCORES)))
    out = np.stack([res.results[b]["out"] for b in range(NCORES)], axis=0)
    return out.astype(np.float32)


# revision 21
# speedup vs baseline: 1.0252x; 1.0201x over previous
"""Multi-head attention (B=8, N=1024, D=768, H=12) on 8 TRN2 NeuronCores.

Strategy: data-parallel over batch — core b computes the full attention for
batch element b. No collectives. Inside a core everything is laid out so no
on-chip transposes are needed:

  - host feeds x[b] pre-transposed as xT [768, 1024] (feature-major, bf16)
  - qT = W_q-as-lhsT @ xT   -> [768, 1024]  (feature-major, per-head rows)
  - kT likewise; v = x @ W_v  -> [1024, 768] (token-major), stored per-head
    as [ki, head, 65] with a ones column at col 64
  - S^T tile = kT_h-slice @ qT_h -> [ki, qi]: softmax-exp is elementwise on
    ACT, and the PV matmul consumes exp(S^T) directly as the moving operand:
      out^T[d(+denom), qi] = v_aug.T @ E — the ones column makes row 64 the
    softmax denominator, accumulated in the same PSUM tile.
  - normalize via reciprocal + partition-broadcast (DRAM-bounce DMA) multiply
  - proj uses attn_out^T tiles as stationary, W_proj natural-layout moving.

Matmuls run in bf16 (inputs rounded host-side / on-chip), accumulation fp32.
"""

import numpy as np
import ml_dtypes

B = 8
N = 1024
D = 768
H = 12
HD = 64
P = 128
DT = 6  # number of 128-row tiles in D
NT = 8  # number of 128-row tiles in N
NCORES = 8

BF16 = ml_dtypes.bfloat16

_CACHE = {}


def _emit(nc, tc, tile, mybir, dram, rep=0):
    import contextlib

    import concourse.bass as bass

    f32 = mybir.dt.float32
    bf16 = mybir.dt.bfloat16
    Exp = mybir.ActivationFunctionType.Exp

    xT_d, wqkv_d, bqk_d, bvb_d, wproj_d, bpb_d, out_d = dram
    # DRAM bounce for partition-broadcasting softmax reciprocal denominators
    # (SBUF->SBUF DMA cannot read with partition step 0; DRAM can).
    rds_d = nc.dram_tensor(f"rdscratch{rep}", [DT, 4, 512], f32)

    ctx = contextlib.ExitStack()
    with ctx:
        persist = ctx.enter_context(tc.tile_pool(name="persist", bufs=1))
        wqk_pool = ctx.enter_context(tc.tile_pool(name="wqk", bufs=4))
        wbig_pool = ctx.enter_context(tc.tile_pool(name="wbig", bufs=2))
        e_pool = ctx.enter_context(tc.tile_pool(name="e", bufs=3))
        bc_pool = ctx.enter_context(tc.tile_pool(name="bc", bufs=4))
        rd_pool = ctx.enter_context(tc.tile_pool(name="rd", bufs=4))
        ost_pool = ctx.enter_context(tc.tile_pool(name="ost", bufs=3))
        # PSUM budget (8 banks): qkv/proj accumulators 2 + S^T double-buffer
        # 2x[128,1024]=4 + PV accumulators 2.
        ps = ctx.enter_context(tc.tile_pool(name="ps", bufs=2, space="PSUM"))
        av = ctx.enter_context(tc.tile_pool(name="av", bufs=2, space="PSUM"))
        sps = ctx.enter_context(tc.tile_pool(name="sps", bufs=2, space="PSUM"))

        xT_t = persist.tile([P, DT, N], bf16)
        qT_t = persist.tile([P, DT, N], bf16)
        kT_t = persist.tile([P, DT, N], bf16)
        vst_t = persist.tile([P, NT, H, HD + 1], bf16)
        aT_t = persist.tile([P, DT, N], bf16)
        bqk_t = persist.tile([P, 2 * DT], f32)
        bvb_t = persist.tile([P, D], f32)
        bpb_t = persist.tile([P, D], f32)

        # input loads
        nc.sync.dma_start(out=xT_t[:], in_=xT_d.ap().rearrange("(i p) n -> p i n", p=P))
        nc.sync.dma_start(out=bqk_t[:], in_=bqk_d[:, :])
        nc.sync.dma_start(out=bvb_t[:], in_=bvb_d[:, :])
        nc.sync.dma_start(out=bpb_t[:], in_=bpb_d[:, :])
        wv_t = wbig_pool.tile([P, DT, D], bf16, tag="wbig")
        nc.sync.dma_start(
            out=wv_t[:],
            in_=wqkv_d.ap()[:, 2 * D : 3 * D].rearrange("(i p) f -> p i f", p=P),
        )
        wp_t = wbig_pool.tile([P, DT, D], bf16, tag="wbig", name="wp_t")
        nc.sync.dma_start(
            out=wp_t[:], in_=wproj_d.ap().rearrange("(i p) f -> p i f", p=P)
        )
        nc.vector.memset(vst_t[:, :, :, HD : HD + 1], 1.0)

        def emit_qk(pr):
            for woff, boff, dstT in ((0, 0, qT_t), (D, DT, kT_t)):
                wt = wqk_pool.tile([P, DT, P], bf16, tag="wqk", name=f"wt{pr}_{boff}")
                nc.sync.dma_start(
                    out=wt[:],
                    in_=wqkv_d.ap()[
                        :, woff + pr * P : woff + (pr + 1) * P
                    ].rearrange("(i p) f -> p i f", p=P),
                )
                for nh in range(2):
                    pst = ps.tile([P, 512], f32, tag="ps", name=f"qkp{pr}_{boff}_{nh}")
                    for d in range(DT):
                        nc.tensor.matmul(
                            pst[:],
                            lhsT=wt[:, d, :],
                            rhs=xT_t[:, d, nh * 512 : (nh + 1) * 512],
                            start=(d == 0),
                            stop=(d == DT - 1),
                        )
                    nc.vector.tensor_scalar_add(
                        out=dstT[:, pr, nh * 512 : (nh + 1) * 512],
                        in0=pst[:],
                        scalar1=bqk_t[:, pr + boff : pr + boff + 1],
                    )

        def emit_v(nt):
            for c0, cw in ((0, 512), (512, 256)):
                pst = ps.tile([P, 512], f32, tag="ps", name=f"vp{nt}_{c0}")
                for d in range(DT):
                    nc.tensor.matmul(
                        pst[:, :cw],
                        lhsT=xT_t[:, d, nt * P : (nt + 1) * P],
                        rhs=wv_t[:, d, c0 : c0 + cw],
                        start=(d == 0),
                        stop=(d == DT - 1),
                    )
                nc.vector.tensor_add(
                    out=vst_t[:, nt, c0 // HD : (c0 + cw) // HD, 0:HD],
                    in0=pst[:, :cw].rearrange("p (h c) -> p h c", c=HD),
                    in1=bvb_t[:, c0 : c0 + cw].rearrange("p (h c) -> p h c", c=HD),
                )

        def emit_attn(pr, nh):
            # S^T matmuls for the two heads land on disjoint PE row groups
            # (partitions 0:64 / 64:128) so they run concurrently on the array.
            qs = slice(nh * 512, (nh + 1) * 512)
            avts = [
                av.tile([P, 512], f32, tag="av", name=f"avt{pr}_{nh}_{j}")
                for j in range(2)
            ]
            for ki in range(NT):
                spt = sps.tile([P, 1024], f32, tag="sps", name=f"spt{pr}_{nh}_{ki}")
                for j, hp in enumerate((0, HD)):
                    nc.tensor.matmul(
                        spt[:, j * 512 : (j + 1) * 512],
                        lhsT=kT_t[hp : hp + HD, pr, ki * P : (ki + 1) * P],
                        rhs=qT_t[hp : hp + HD, pr, qs],
                        start=True,
                        stop=True,
                    )
                et = e_pool.tile([P, 1024], bf16, tag="e", name=f"et{pr}_{nh}_{ki}")
                nc.scalar.activation(out=et[:], in_=spt[:], func=Exp, scale=0.125)
                for j in range(2):
                    nc.tensor.matmul(
                        avts[j][0 : HD + 1, :],
                        lhsT=vst_t[:, ki, 2 * pr + j, :],
                        rhs=et[:, j * 512 : (j + 1) * 512],
                        start=(ki == 0),
                        stop=(ki == NT - 1),
                    )
            for j in range(2):
                hp = j * HD
                rdt = rd_pool.tile([1, 512], f32, tag="rd", name=f"rdt{pr}_{nh}_{j}")
                nc.vector.reciprocal(rdt[0:1, :], avts[j][HD : HD + 1, :])
                nc.sync.dma_start(out=rds_d.ap()[pr, 2 * nh + j, :], in_=rdt[0:1, :])
                bct = bc_pool.tile([HD, 512], f32, tag="bc", name=f"bct{pr}_{nh}_{j}")
                src = rds_d.ap()[pr, 2 * nh + j, :]
                src_b = bass.AP(
                    tensor=src.tensor, offset=src.offset, ap=[[0, HD]] + list(src.ap)
                )
                nc.sync.dma_start(out=bct[:], in_=src_b)
                nc.vector.tensor_mul(
                    out=aT_t[hp : hp + HD, pr, qs],
                    in0=avts[j][0:HD, :],
                    in1=bct[:, :],
                )

        def emit_proj(nt):
            ot = ost_pool.tile([P, D], f32, tag="ost", name=f"ot{nt}")
            for c0, cw in ((0, 512), (512, 256)):
                pst = ps.tile([P, 512], f32, tag="ps", name=f"pp{nt}_{c0}")
                for d in range(DT):
                    nc.tensor.matmul(
                        pst[:, :cw],
                        lhsT=aT_t[:, d, nt * P : (nt + 1) * P],
                        rhs=wp_t[:, d, c0 : c0 + cw],
                        start=(d == 0),
                        stop=(d == DT - 1),
                    )
                nc.vector.tensor_add(
                    out=ot[:, c0 : c0 + cw],
                    in0=pst[:, :cw],
                    in1=bpb_t[:, c0 : c0 + cw],
                )
            nc.sync.dma_start(out=out_d.ap()[nt * P : (nt + 1) * P, :], in_=ot[:])

        # phase ordering: qk(0) unblocks attention earliest; V next (vst is
        # consumed per-ki so early tiles matter); remaining qk pairs; then
        # attention with qi-half outer so the first half of proj can overlap
        # the second half of attention.
        emit_qk(0)
        for nt in range(NT):
            emit_v(nt)
        for pr in range(1, DT):
            emit_qk(pr)
        for pr in range(DT):
            emit_attn(pr, 0)
        for nt in range(NT // 2):
            emit_proj(nt)
        for pr in range(DT):
            emit_attn(pr, 1)
        for nt in range(NT // 2, NT):
            emit_proj(nt)


def build_module(reps=1, barrier=False):
    import concourse.mybir as mybir
    import concourse.tile as tile
    from concourse import bacc

    f32 = mybir.dt.float32
    bf16 = mybir.dt.bfloat16
    nc = bacc.Bacc(None, target_bir_lowering=False)
    xT_d = nc.dram_tensor("xT", [D, N], bf16, kind="ExternalInput")
    wqkv_d = nc.dram_tensor("w_qkv", [D, 3 * D], bf16, kind="ExternalInput")
    bqk_d = nc.dram_tensor("b_qk", [P, 2 * DT], f32, kind="ExternalInput")
    bvb_d = nc.dram_tensor("b_v_bc", [P, D], f32, kind="ExternalInput")
    wproj_d = nc.dram_tensor("w_proj", [D, D], bf16, kind="ExternalInput")
    bpb_d = nc.dram_tensor("b_proj_bc", [P, D], f32, kind="ExternalInput")
    out_d = nc.dram_tensor("out", [N, D], f32, kind="ExternalOutput")

    with tile.TileContext(nc) as tc:
        for rep in range(reps):
            if barrier and rep:
                tc.strict_bb_all_engine_barrier()
            _emit(
                nc,
                tc,
                tile,
                mybir,
                (xT_d, wqkv_d, bqk_d, bvb_d, wproj_d, bpb_d, out_d),
                rep=rep,
            )
    nc.compile()
    return nc


def get_nc():
    if "nc" not in _CACHE:
        _CACHE["nc"] = build_module()
    return _CACHE["nc"]


def shard_inputs(x, W_qkv, b_qkv, W_proj, b_proj):
    """Host-side layout/dtype prep + per-core sharding (batch data-parallel)."""
    x = np.asarray(x, dtype=np.float32)
    W_qkv = np.asarray(W_qkv, dtype=np.float32)
    b_qkv = np.asarray(b_qkv, dtype=np.float32)
    W_proj = np.asarray(W_proj, dtype=np.float32)
    b_proj = np.asarray(b_proj, dtype=np.float32)

    xT = np.ascontiguousarray(x.transpose(0, 2, 1)).astype(BF16)  # [B, D, N]
    wqkv = W_qkv.astype(BF16)
    wproj = W_proj.astype(BF16)
    bqk = np.ascontiguousarray(b_qkv[: 2 * D].reshape(2 * DT, P).T)  # [P, 12]
    bvb = np.ascontiguousarray(np.broadcast_to(b_qkv[2 * D :], (P, D)))
    bpb = np.ascontiguousarray(np.broadcast_to(b_proj, (P, D)))
    return [
        {
            "xT": xT[b],
            "w_qkv": wqkv,
            "b_qk": bqk,
            "b_v_bc": bvb,
            "w_proj": wproj,
            "b_proj_bc": bpb,
        }
        for b in range(NCORES)
    ]


def kernel(x, W_qkv, b_qkv, W_proj, b_proj):
    from concourse.bass_utils import run_bass_kernel_spmd

    nc = get_nc()
    in_maps = shard_inputs(x, W_qkv, b_qkv, W_proj, b_proj)
    res = run_bass_kernel_spmd(nc, in_maps, list(range(NCORES)))
    out = np.stack([res.results[b]["out"] for b in range(NCORES)], axis=0)
    return out.astype(np.float32)
